# revision 24
# baseline (speedup 1.0000x reference)
"""Multi-head dot-product attention (single query step) on 8 TRN2 NeuronCores.

Reference computation (B=64, S=2048, QD=1024, KD=512, AD=512, H=8):
    q      = einsum('bq,haq->bha', query, Wq)            # (B,H,AD)
    k      = einsum('bsk,hak->bhsa', keys, Wk)           # (B,H,S,AD)
    energy = einsum('bha,bhsa->bhs', q, k)               # (B,H,S)
    energy = where(s >= key_len[b], -1e10, energy)
    attn   = softmax(energy, axis=-1)                    # (B,H,S)
    ctx    = einsum('bhs,bsd->bhd', attn, value)         # (B,H,KD) -> (B, H*KD)
    return ctx.reshape(B, -1), attn

Key algebraic optimization: energy contracts q against k over AD, so Wk folds
into the query side:
    qk[b,h,c] = sum_a q[b,h,a] * Wk[h,a,c]               # (B,H,KD) - tiny
    energy[b,h,s] = sum_c qk[b,h,c] * keys[b,s,c]
This removes the (B,H,S,AD) key projection (~550 GFLOP) entirely; the kernel
becomes memory-bound on streaming keys+value once each.

Sharding: attention is data-parallel over batch (8 batches per core). The
tiny qk projection is sharded over HEADS instead (core h computes head h for
all 64 batches, so each core loads only its own Wq[h]/Wk[h] - 3MB instead of
24MB of weights), then a 128KB-per-core AllToAll redistributes qk so every
core holds all heads for its own batches.

Matmuls run in float32r (single-pass fp32, ~1.5e-4 rel err) - 4x the
streaming rate of strict fp32 on the PE.

Masking: one fused DVE op per energy tile computes
    energy' = min(base[s] + key_len[b]*1e7, energy),  base[s] = 1e6-(s+1)*1e7
valid positions keep energy exactly; masked positions become <= -9e6 whose
exp underflows to exactly 0 - same as the reference's -1e10 masked_fill.
"""

import numpy as np

import concourse.bass as bass
import concourse.bacc as bacc
import concourse.mybir as mybir
from concourse.tile import TileContext
from concourse.masks import make_identity
from concourse.bass_utils import run_bass_kernel_spmd
from contextlib import ExitStack

FP = mybir.dt.float32
FPR = mybir.dt.float32r
P = 128

B, S, QD, KD, AD, H = 64, 2048, 1024, 512, 512, 8
NCORES = 8
BL = B // NCORES          # batches per core = 8
BH = BL * H               # (b,h) rows per core = 64
QC = QD // P              # 8 chunks of query dim
AC = AD // P              # 4 chunks of attention dim
CC = KD // P              # 4 chunks of key-feature dim
SC = S // P               # 16 chunks of seq dim
SH = S // 2               # seq half


def build_kernel():
    nc = bacc.Bacc("TRN2", target_bir_lowering=False, debug=False, num_devices=NCORES)

    # All inputs are pre-laid-out on the host so every DMA is a contiguous
    # [128, X] block per partition. wq/wk hold only THIS core's head.
    queryt = nc.declare_dram_parameter("queryt", [P, QC, B], FPR, isOutput=False)
    wq = nc.declare_dram_parameter("wq", [P, QC, AD], FPR, isOutput=False)
    wk = nc.declare_dram_parameter("wk", [P, AC, KD], FPR, isOutput=False)
    keyst = nc.declare_dram_parameter("keyst", [BL, 2, P, CC, SH], FPR, isOutput=False)
    value = nc.declare_dram_parameter(
        "value", [BL, 2, P, SC // 2, KD], FPR, isOutput=False
    )
    base = nc.declare_dram_parameter("base", [H, S], FP, isOutput=False)
    klen7 = nc.declare_dram_parameter("klen7", [H, BL], FP, isOutput=False)
    attn_out = nc.declare_dram_parameter("attn", [BH, S], FP, isOutput=True)
    ctx_out = nc.declare_dram_parameter("ctx", [BH, KD], FP, isOutput=True)

    with TileContext(nc) as tc, ExitStack() as ctx:
        persist = ctx.enter_context(tc.tile_pool(name="persist", bufs=1))
        dram = ctx.enter_context(tc.tile_pool(name="dram", bufs=1, space="DRAM"))
        en_pool = ctx.enter_context(tc.tile_pool(name="en", bufs=4))
        stats = ctx.enter_context(tc.tile_pool(name="st", bufs=3))
        ctxo_pool = ctx.enter_context(tc.tile_pool(name="cx", bufs=2))
        keys_pool = ctx.enter_context(tc.tile_pool(name="keys", bufs=5))
        val_pool = ctx.enter_context(tc.tile_pool(name="val", bufs=4))
        psum_small = ctx.enter_context(tc.tile_pool(name="ps", bufs=3, space="PSUM"))
        psum_energy = ctx.enter_context(tc.tile_pool(name="pe", bufs=2, space="PSUM"))

        # ---- warm-up collective: absorbs cross-core launch skew and ncfw
        # cold-start while the input DMAs prefetch, so the real qk exchange
        # below is cheap ----
        dummy_in = dram.tile([NCORES, 32], FP)
        dummy_out = dram.tile([NCORES, 32], FP)
        nc.gpsimd.collective_compute(
            "AllToAll",
            mybir.AluOpType.bypass,
            ins=[dummy_in[:].opt()],
            outs=[dummy_out[:].opt()],
            replica_groups=[list(range(NCORES))],
        )

        # ---- constants / persistent tiles ----
        ident = persist.tile([64, 64], FP)
        make_identity(nc, ident)
        queryt_sb = persist.tile([P, QC, B], FPR)
        nc.sync.dma_start(queryt_sb[:], queryt[:])
        # weights borrow value-pool slots (released after phase 1); DMAs are
        # split so they spread over several DMA queues and finish fast
        wq_sb = val_pool.tile([P, QC, AD], FPR, tag="v")
        for i in range(4):
            nc.sync.dma_start(
                wq_sb[:, 2 * i:2 * i + 2, :], wq[:, 2 * i:2 * i + 2, :]
            )
        wk_sb = val_pool.tile([P, AC, KD], FPR, tag="v")
        for i in range(2):
            nc.sync.dma_start(
                wk_sb[:, 2 * i:2 * i + 2, :], wk[:, 2 * i:2 * i + 2, :]
            )
        base_sb = persist.tile([H, S], FP)
        nc.sync.dma_start(base_sb[:], base[:])
        klen7_sb = persist.tile([H, BL], FP)
        nc.sync.dma_start(klen7_sb[:], klen7[:])

        qt_sb = persist.tile([P, AC, B], FPR)        # q.T[a, b] (this head)
        qkt_loc = persist.tile([P, NCORES, CC, BL], FPR)  # qk.T[c, b] (this head)
        qkt_sb = persist.tile([P, H, CC, BL], FPR)   # gathered qk.T[c, h, b_local]
        attnt_sb = persist.tile([P, SC, BH], FPR)    # attn.T[s, (b,h)]
        q_sb = persist.tile([B, AD], FP)
        qk_sb = persist.tile([B, KD], FP)
        a2a_in = dram.tile([NCORES, P, CC, BL], FPR)
        a2a_out = dram.tile([NCORES, P, CC, BL], FPR)

        # ---- phase 1: q = query @ Wq[h]^T for ALL batches (this head) ----
        q_ps = psum_small.tile([B, AD], FP, tag="sm")
        for qc in range(QC):
            nc.tensor.matmul(
                q_ps[:],
                queryt_sb[:, qc, :],       # lhsT [K=128, M=64]
                wq_sb[:, qc, :],           # rhs  [K=128, N=512]
                start=(qc == 0),
                stop=(qc == QC - 1),
            )
        nc.scalar.copy(q_sb[:], q_ps[:])
        tr_ps = psum_small.tile([P, AC, B], FP, tag="sm")
        for ac in range(AC):
            nc.tensor.transpose(
                tr_ps[:, ac, :], q_sb[:, ac * P:(ac + 1) * P], ident[:]
            )
        nc.vector.tensor_copy(qt_sb[:], tr_ps[:])

        # ---- qk = q @ Wk[h] for ALL batches (this head) ----
        qk_ps = psum_small.tile([B, KD], FP, tag="sm")
        for ac in range(AC):
            nc.tensor.matmul(
                qk_ps[:],
                qt_sb[:, ac, :],           # lhsT [K=128, M=64]
                wk_sb[:, ac, :],           # rhs  [K=128, N=512]
                start=(ac == 0),
                stop=(ac == AC - 1),
            )
        nc.scalar.copy(qk_sb[:], qk_ps[:])
        tr_ps2 = psum_small.tile([P, CC, B], FP, tag="sm")
        for cc in range(CC):
            nc.tensor.transpose(
                tr_ps2[:, cc, :], qk_sb[:, cc * P:(cc + 1) * P], ident[:]
            )
        # write dest-rank-major so the a2a_in DMA has 128B-contiguous runs
        nc.vector.tensor_copy(
            qkt_loc[:].rearrange("p d cc b -> p cc d b"), tr_ps2[:]
        )

        # ---- AllToAll: core h sends qk.T[head h, batches of core j] to j ----
        nc.sync.dma_start(
            a2a_in[:].rearrange("d p cc b -> p d cc b"), qkt_loc[:]
        )
        nc.gpsimd.collective_compute(
            "AllToAll",
            mybir.AluOpType.bypass,
            ins=[a2a_in[:].opt()],
            outs=[a2a_out[:].opt()],
            replica_groups=[list(range(NCORES))],
        )
        nc.sync.dma_start(
            qkt_sb[:], a2a_out[:].rearrange("h p cc b -> p h cc b")
        )

        # ---- per-batch pipeline: energy -> mask -> softmax -> transpose -> ctx
        en_tiles = {}

        def energy(b):
            en_sb = en_pool.tile([H, S], FP, tag="en")
            en_tiles[b] = en_sb
            for half in range(2):
                kt_sb = keys_pool.tile([P, CC, SH], FPR, tag="kt")
                nc.sync.dma_start(kt_sb[:], keyst[b, half])
                en_ps = psum_energy.tile([H, SH], FP, tag="enp")
                for sc2 in range(2):
                    for cc in range(CC):
                        nc.tensor.matmul(
                            en_ps[:, sc2 * 512:(sc2 + 1) * 512],
                            qkt_sb[:, :, cc, b],             # lhsT [K=128, M=8] (h)
                            kt_sb[:, cc, sc2 * 512:(sc2 + 1) * 512],
                            start=(cc == 0),
                            stop=(cc == CC - 1),
                        )
                # fused mask + psum evict:
                # en = min(base[s] + klen[b]*1e7, energy)
                nc.vector.scalar_tensor_tensor(
                    en_sb[:, half * SH:(half + 1) * SH],
                    base_sb[:, half * SH:(half + 1) * SH],
                    klen7_sb[:, b:b + 1],
                    en_ps[:],
                    op0=mybir.AluOpType.add,
                    op1=mybir.AluOpType.min,
                )

        rsums = {}
        rinvs = {}

        def exp_block(b):
            # unnormalized softmax numerator; energies are O(+-45) so exp
            # cannot overflow f32 and the max-subtraction can be skipped
            # (mathematically identical softmax)
            en_sb = en_tiles[b]
            rsum = stats.tile([H, 1], FP, tag="rs")
            rsums[b] = rsum
            nc.scalar.activation(
                en_sb[:], en_sb[:], mybir.ActivationFunctionType.Exp,
                bias=0.0, accum_out=rsum[:],
            )

        def recip_block(b):
            rinv = stats.tile([H, 1], FP, tag="ri")
            rinvs[b] = rinv
            nc.vector.reciprocal(rinv[:], rsums[b][:])

        def norm_block(b):
            # normalize the attn output in place AFTER the transposes have
            # read the unnormalized exp (Tile's WAR dep orders this); ctx
            # folds 1/rsum into its psum evict instead
            en_sb = en_tiles[b]
            nc.vector.tensor_scalar_mul(en_sb[:], en_sb[:], rinvs[b][:])
            nc.sync.dma_start(attn_out[b * H:(b + 1) * H, :], en_sb[:])

        def transposes(b):
            en_sb = en_tiles[b]
            for g in range(4):
                tr_ps = psum_small.tile([P, 4, H], FP, tag="sm")
                for j in range(4):
                    sc = g * 4 + j
                    nc.tensor.transpose(
                        tr_ps[:, j, :], en_sb[:, sc * P:(sc + 1) * P],
                        ident[0:8, 0:8],
                    )
                nc.vector.tensor_copy(
                    attnt_sb[:, g * 4:(g + 1) * 4, b * H:(b + 1) * H], tr_ps[:]
                )

        def ctx_block(b):
            ctx_ps = psum_small.tile([BL, KD], FP, tag="sm")
            for half in range(2):
                v_sb = val_pool.tile([P, SC // 2, KD], FPR, tag="v")
                nc.sync.dma_start(v_sb[:], value[b, half])
                for sc2 in range(SC // 2):
                    sc = half * (SC // 2) + sc2
                    nc.tensor.matmul(
                        ctx_ps[:],
                        attnt_sb[:, sc, b * H:(b + 1) * H],   # lhsT [K=128, M=8]
                        v_sb[:, sc2, :],                       # rhs  [K=128, N=512]
                        start=(sc == 0),
                        stop=(sc == SC - 1),
                    )
            ctx_sb = ctxo_pool.tile([BL, KD], FP, tag="c")
            # evict with the softmax normalization folded in
            nc.scalar.mul(ctx_sb[:], ctx_ps[:], rinvs[b][:])
            nc.sync.dma_start(ctx_out[b * H:(b + 1) * H, :], ctx_sb[:])

        energy(0)
        energy(1)
        exp_block(0)
        for b in range(BL):
            if b + 1 < BL:
                exp_block(b + 1)
            recip_block(b)
            transposes(b)
            ctx_block(b)
            norm_block(b)
            if b + 2 < BL:
                energy(b + 2)

    nc.compile()
    return nc


_NC = None


def _get_nc():
    global _NC
    if _NC is None:
        _NC = build_kernel()
    return _NC


def _make_base():
    s = np.arange(S, dtype=np.float64)
    row = (1e6 - (s + 1) * 1e7).astype(np.float32)
    return np.ascontiguousarray(np.tile(row, (H, 1)))


def _prep_core_inputs(query_t, keys, value, key_len, wqt_h, wk_h, base_row, c):
    bs = slice(c * BL, (c + 1) * BL)
    # keyst: [BL, half, P, CC, SH] so each half-tile DMA is fully contiguous
    kt = keys[bs].transpose(0, 2, 1).reshape(BL, CC, P, 2, SH)
    kt = np.ascontiguousarray(kt.transpose(0, 3, 2, 1, 4))
    # value: [BL, half, P, SC//2, KD]
    v = value[bs].reshape(BL, 2, SC // 2, P, KD)
    v = np.ascontiguousarray(v.transpose(0, 1, 3, 2, 4))
    kl = (key_len[bs].astype(np.float64) * 1e7).astype(np.float32)
    kl = np.ascontiguousarray(np.tile(kl[None, :], (H, 1)))
    return {
        "queryt": query_t, "wq": wqt_h[c], "wk": wk_h[c],
        "keyst": kt, "value": v, "base": base_row, "klen7": kl,
    }


def make_in_maps(query, keys, value, key_len, Wq, Wk):
    query = np.asarray(query, dtype=np.float32)
    keys = np.asarray(keys, dtype=np.float32)
    value = np.asarray(value, dtype=np.float32)
    key_len = np.asarray(key_len, dtype=np.int32)
    Wq = np.asarray(Wq, dtype=np.float32)
    Wk = np.asarray(Wk, dtype=np.float32)

    query_t = np.ascontiguousarray(query.T.reshape(QC, P, B).transpose(1, 0, 2))
    wqt_h = np.ascontiguousarray(
        Wq.transpose(0, 2, 1).reshape(H, QC, P, AD).transpose(0, 2, 1, 3)
    )
    wk_h = np.ascontiguousarray(
        Wk.reshape(H, AC, P, KD).transpose(0, 2, 1, 3)
    )
    base_row = _make_base()

    return [
        _prep_core_inputs(query_t, keys, value, key_len, wqt_h, wk_h, base_row, c)
        for c in range(NCORES)
    ]


def kernel(query, keys, value, key_len, Wq, Wk):
    in_maps = make_in_maps(query, keys, value, key_len, Wq, Wk)
    nc = _get_nc()
    results = run_bass_kernel_spmd(nc, in_maps, core_ids=list(range(NCORES))).results

    ctx = np.concatenate([r["ctx"].reshape(BL, H * KD) for r in results], axis=0)
    attn = np.concatenate(
        [r["attn"].reshape(BL, H, S) for r in results], axis=0
    )
    return ctx, attn


# revision 25
# speedup vs baseline: 1.0815x; 1.0815x over previous
"""Multi-head dot-product attention (single query step) on 8 TRN2 NeuronCores.

Reference computation (B=64, S=2048, QD=1024, KD=512, AD=512, H=8):
    q      = einsum('bq,haq->bha', query, Wq)            # (B,H,AD)
    k      = einsum('bsk,hak->bhsa', keys, Wk)           # (B,H,S,AD)
    energy = einsum('bha,bhsa->bhs', q, k)               # (B,H,S)
    energy = where(s >= key_len[b], -1e10, energy)
    attn   = softmax(energy, axis=-1)                    # (B,H,S)
    ctx    = einsum('bhs,bsd->bhd', attn, value)         # (B,H,KD) -> (B, H*KD)
    return ctx.reshape(B, -1), attn

Key algebraic optimization: energy contracts q against k over AD, so Wk folds
into the query side:
    qk[b,h,c] = sum_a q[b,h,a] * Wk[h,a,c]               # (B,H,KD) - tiny
    energy[b,h,s] = sum_c qk[b,h,c] * keys[b,s,c]
This removes the (B,H,S,AD) key projection (~550 GFLOP) entirely; the kernel
becomes memory-bound on streaming keys+value once each.

Sharding: attention is data-parallel over batch (8 batches per core). The
tiny qk projection is sharded over HEADS instead (core h computes head h for
all 64 batches, so each core loads only its own Wq[h]/Wk[h] - 3MB instead of
24MB of weights), then a 128KB-per-core AllToAll redistributes qk so every
core holds all heads for its own batches.

Matmuls run in float32r (single-pass fp32, ~1.5e-4 rel err) - 4x the
streaming rate of strict fp32 on the PE.

Masking: one fused DVE op per energy tile computes
    energy' = min(base[s] + key_len[b]*1e7, energy),  base[s] = 1e6-(s+1)*1e7
valid positions keep energy exactly; masked positions become <= -9e6 whose
exp underflows to exactly 0 - same as the reference's -1e10 masked_fill.
"""

import numpy as np

import concourse.bass as bass
import concourse.bacc as bacc
import concourse.mybir as mybir
from concourse.tile import TileContext
from concourse.masks import make_identity
from concourse.bass_utils import run_bass_kernel_spmd
from contextlib import ExitStack

FP = mybir.dt.float32
FPR = mybir.dt.float32r
P = 128

B, S, QD, KD, AD, H = 64, 2048, 1024, 512, 512, 8
NCORES = 8
BL = B // NCORES          # batches per core = 8
BH = BL * H               # (b,h) rows per core = 64
QC = QD // P              # 8 chunks of query dim
AC = AD // P              # 4 chunks of attention dim
CC = KD // P              # 4 chunks of key-feature dim
SC = S // P               # 16 chunks of seq dim
SH = S // 2               # seq half


def build_kernel():
    nc = bacc.Bacc("TRN2", target_bir_lowering=False, debug=False, num_devices=NCORES)

    # All inputs are pre-laid-out on the host so every DMA is a contiguous
    # [128, X] block per partition. wq/wk hold only THIS core's head.
    queryt = nc.declare_dram_parameter("queryt", [P, QC, B], FPR, isOutput=False)
    wq = nc.declare_dram_parameter("wq", [P, QC, AD], FPR, isOutput=False)
    wk = nc.declare_dram_parameter("wk", [P, AC, KD], FPR, isOutput=False)
    keyst = nc.declare_dram_parameter("keyst", [BL, 2, P, CC, SH], FPR, isOutput=False)
    value = nc.declare_dram_parameter(
        "value", [BL, 2, P, SC // 2, KD], FPR, isOutput=False
    )
    base = nc.declare_dram_parameter("base", [H, S], FP, isOutput=False)
    klen7 = nc.declare_dram_parameter("klen7", [H, BL], FP, isOutput=False)
    attn_out = nc.declare_dram_parameter("attn", [BH, S], FP, isOutput=True)
    ctx_out = nc.declare_dram_parameter("ctx", [BH, KD], FP, isOutput=True)

    with TileContext(nc) as tc, ExitStack() as ctx:
        persist = ctx.enter_context(tc.tile_pool(name="persist", bufs=1))
        dram = ctx.enter_context(tc.tile_pool(name="dram", bufs=1, space="DRAM"))
        en_pool = ctx.enter_context(tc.tile_pool(name="en", bufs=4))
        stats = ctx.enter_context(tc.tile_pool(name="st", bufs=3))
        ctxo_pool = ctx.enter_context(tc.tile_pool(name="cx", bufs=2))
        keys_pool = ctx.enter_context(tc.tile_pool(name="keys", bufs=5))
        val_pool = ctx.enter_context(tc.tile_pool(name="val", bufs=4))
        psum_small = ctx.enter_context(tc.tile_pool(name="ps", bufs=3, space="PSUM"))
        psum_energy = ctx.enter_context(tc.tile_pool(name="pe", bufs=2, space="PSUM"))

        # ---- warm-up collective: absorbs cross-core launch skew and ncfw
        # cold-start while the input DMAs prefetch, so the real qk exchange
        # below is cheap ----
        dummy_in = dram.tile([NCORES, 32], FP)
        dummy_out = dram.tile([NCORES, 32], FP)
        nc.gpsimd.collective_compute(
            "AllToAll",
            mybir.AluOpType.bypass,
            ins=[dummy_in[:].opt()],
            outs=[dummy_out[:].opt()],
            replica_groups=[list(range(NCORES))],
        )

        # ---- constants / persistent tiles ----
        ident = persist.tile([64, 64], FP)
        make_identity(nc, ident)
        queryt_sb = persist.tile([P, QC, B], FPR)
        nc.sync.dma_start(queryt_sb[:], queryt[:])
        # weights borrow value-pool slots (released after phase 1); DMAs are
        # split so they spread over several DMA queues and finish fast
        wq_sb = val_pool.tile([P, QC, AD], FPR, tag="v")
        for i in range(4):
            nc.sync.dma_start(
                wq_sb[:, 2 * i:2 * i + 2, :], wq[:, 2 * i:2 * i + 2, :]
            )
        wk_sb = val_pool.tile([P, AC, KD], FPR, tag="v")
        for i in range(2):
            nc.sync.dma_start(
                wk_sb[:, 2 * i:2 * i + 2, :], wk[:, 2 * i:2 * i + 2, :]
            )
        base_sb = persist.tile([H, S], FP)
        nc.sync.dma_start(base_sb[:], base[:])
        klen7_sb = persist.tile([H, BL], FP)
        nc.sync.dma_start(klen7_sb[:], klen7[:])

        qt_sb = persist.tile([P, AC, B], FPR)        # q.T[a, b] (this head)
        qkt_loc = persist.tile([P, NCORES, CC, BL], FPR)  # qk.T[c, b] (this head)
        qkt_sb = persist.tile([P, H, CC, BL], FPR)   # gathered qk.T[c, h, b_local]
        attnt_sb = persist.tile([P, SC, BH], FPR)    # attn.T[s, (b,h)]
        q_sb = persist.tile([B, AD], FP)
        qk_sb = persist.tile([B, KD], FP)
        a2a_in = dram.tile([NCORES, P, CC, BL], FPR)
        a2a_out = dram.tile([NCORES, P, CC, BL], FPR)

        # ---- phase 1: q = query @ Wq[h]^T for ALL batches (this head) ----
        q_ps = psum_small.tile([B, AD], FP, tag="sm")
        for qc in range(QC):
            nc.tensor.matmul(
                q_ps[:],
                queryt_sb[:, qc, :],       # lhsT [K=128, M=64]
                wq_sb[:, qc, :],           # rhs  [K=128, N=512]
                start=(qc == 0),
                stop=(qc == QC - 1),
            )
        nc.scalar.copy(q_sb[:], q_ps[:])
        tr_ps = psum_small.tile([P, AC, B], FP, tag="sm")
        for ac in range(AC):
            nc.tensor.transpose(
                tr_ps[:, ac, :], q_sb[:, ac * P:(ac + 1) * P], ident[:]
            )
        nc.vector.tensor_copy(qt_sb[:], tr_ps[:])

        # ---- qk = q @ Wk[h] for ALL batches (this head) ----
        qk_ps = psum_small.tile([B, KD], FP, tag="sm")
        for ac in range(AC):
            nc.tensor.matmul(
                qk_ps[:],
                qt_sb[:, ac, :],           # lhsT [K=128, M=64]
                wk_sb[:, ac, :],           # rhs  [K=128, N=512]
                start=(ac == 0),
                stop=(ac == AC - 1),
            )
        nc.scalar.copy(qk_sb[:], qk_ps[:])
        tr_ps2 = psum_small.tile([P, CC, B], FP, tag="sm")
        for cc in range(CC):
            nc.tensor.transpose(
                tr_ps2[:, cc, :], qk_sb[:, cc * P:(cc + 1) * P], ident[:]
            )
        # write dest-rank-major so the a2a_in DMA has 128B-contiguous runs
        nc.vector.tensor_copy(
            qkt_loc[:].rearrange("p d cc b -> p cc d b"), tr_ps2[:]
        )

        # ---- AllToAll: core h sends qk.T[head h, batches of core j] to j ----
        nc.sync.dma_start(
            a2a_in[:].rearrange("d p cc b -> p d cc b"), qkt_loc[:]
        )
        nc.gpsimd.collective_compute(
            "AllToAll",
            mybir.AluOpType.bypass,
            ins=[a2a_in[:].opt()],
            outs=[a2a_out[:].opt()],
            replica_groups=[list(range(NCORES))],
        )
        nc.sync.dma_start(
            qkt_sb[:], a2a_out[:].rearrange("h p cc b -> p h cc b")
        )

        # ---- per-batch pipeline: energy -> mask -> softmax -> transpose -> ctx
        en_tiles = {}

        def energy(b):
            en_sb = en_pool.tile([H, S], FP, tag="en")
            en_tiles[b] = en_sb
            for half in range(2):
                kt_sb = keys_pool.tile([P, CC, SH], FPR, tag="kt")
                nc.sync.dma_start(kt_sb[:], keyst[b, half])
                en_ps = psum_energy.tile([H, SH], FP, tag="enp")
                for sc2 in range(2):
                    for cc in range(CC):
                        nc.tensor.matmul(
                            en_ps[:, sc2 * 512:(sc2 + 1) * 512],
                            qkt_sb[:, :, cc, b],             # lhsT [K=128, M=8] (h)
                            kt_sb[:, cc, sc2 * 512:(sc2 + 1) * 512],
                            start=(cc == 0),
                            stop=(cc == CC - 1),
                        )
                # fused mask + psum evict:
                # en = min(base[s] + klen[b]*1e7, energy)
                nc.vector.scalar_tensor_tensor(
                    en_sb[:, half * SH:(half + 1) * SH],
                    base_sb[:, half * SH:(half + 1) * SH],
                    klen7_sb[:, b:b + 1],
                    en_ps[:],
                    op0=mybir.AluOpType.add,
                    op1=mybir.AluOpType.min,
                )

        rsums = {}
        rinvs = {}

        def exp_block(b):
            # unnormalized softmax numerator; energies are O(+-45) so exp
            # cannot overflow f32 and the max-subtraction can be skipped
            # (mathematically identical softmax)
            en_sb = en_tiles[b]
            rsum = stats.tile([H, 1], FP, tag="rs")
            rsums[b] = rsum
            nc.scalar.activation(
                en_sb[:], en_sb[:], mybir.ActivationFunctionType.Exp,
                bias=0.0, accum_out=rsum[:],
            )

        def recip_block(b):
            rinv = stats.tile([H, 1], FP, tag="ri")
            rinvs[b] = rinv
            nc.vector.reciprocal(rinv[:], rsums[b][:])

        def norm_block(b):
            # normalize the attn output in place AFTER the transposes have
            # read the unnormalized exp (Tile's WAR dep orders this); ctx
            # folds 1/rsum into its psum evict instead
            en_sb = en_tiles[b]
            nc.vector.tensor_scalar_mul(en_sb[:], en_sb[:], rinvs[b][:])
            nc.sync.dma_start(attn_out[b * H:(b + 1) * H, :], en_sb[:])

        def transposes(b):
            en_sb = en_tiles[b]
            for g in range(4):
                tr_ps = psum_small.tile([P, 4, H], FP, tag="sm")
                for j in range(4):
                    sc = g * 4 + j
                    nc.tensor.transpose(
                        tr_ps[:, j, :], en_sb[:, sc * P:(sc + 1) * P],
                        ident[0:8, 0:8],
                    )
                nc.vector.tensor_copy(
                    attnt_sb[:, g * 4:(g + 1) * 4, b * H:(b + 1) * H], tr_ps[:]
                )

        def ctx_block(b):
            ctx_ps = psum_small.tile([BL, KD], FP, tag="sm")
            for half in range(2):
                v_sb = val_pool.tile([P, SC // 2, KD], FPR, tag="v")
                nc.sync.dma_start(v_sb[:], value[b, half])
                for sc2 in range(SC // 2):
                    sc = half * (SC // 2) + sc2
                    nc.tensor.matmul(
                        ctx_ps[:],
                        attnt_sb[:, sc, b * H:(b + 1) * H],   # lhsT [K=128, M=8]
                        v_sb[:, sc2, :],                       # rhs  [K=128, N=512]
                        start=(sc == 0),
                        stop=(sc == SC - 1),
                    )
            ctx_sb = ctxo_pool.tile([BL, KD], FP, tag="c")
            # evict with the softmax normalization folded in
            nc.scalar.mul(ctx_sb[:], ctx_ps[:], rinvs[b][:])
            nc.sync.dma_start(ctx_out[b * H:(b + 1) * H, :], ctx_sb[:])

        energy(0)
        energy(1)
        energy(2)
        exp_block(0)
        for b in range(BL):
            if b + 1 < BL:
                exp_block(b + 1)
            recip_block(b)
            transposes(b)
            ctx_block(b)
            if b + 3 < BL:
                energy(b + 3)
            norm_block(b)

    nc.compile()
    return nc


_NC = None


def _get_nc():
    global _NC
    if _NC is None:
        _NC = build_kernel()
    return _NC


def _make_base():
    s = np.arange(S, dtype=np.float64)
    row = (1e6 - (s + 1) * 1e7).astype(np.float32)
    return np.ascontiguousarray(np.tile(row, (H, 1)))


def _prep_core_inputs(query_t, keys, value, key_len, wqt_h, wk_h, base_row, c):
    bs = slice(c * BL, (c + 1) * BL)
    # keyst: [BL, half, P, CC, SH] so each half-tile DMA is fully contiguous
    kt = keys[bs].transpose(0, 2, 1).reshape(BL, CC, P, 2, SH)
    kt = np.ascontiguousarray(kt.transpose(0, 3, 2, 1, 4))
    # value: [BL, half, P, SC//2, KD]
    v = value[bs].reshape(BL, 2, SC // 2, P, KD)
    v = np.ascontiguousarray(v.transpose(0, 1, 3, 2, 4))
    kl = (key_len[bs].astype(np.float64) * 1e7).astype(np.float32)
    kl = np.ascontiguousarray(np.tile(kl[None, :], (H, 1)))
    return {
        "queryt": query_t, "wq": wqt_h[c], "wk": wk_h[c],
        "keyst": kt, "value": v, "base": base_row, "klen7": kl,
    }


def make_in_maps(query, keys, value, key_len, Wq, Wk):
    query = np.asarray(query, dtype=np.float32)
    keys = np.asarray(keys, dtype=np.float32)
    value = np.asarray(value, dtype=np.float32)
    key_len = np.asarray(key_len, dtype=np.int32)
    Wq = np.asarray(Wq, dtype=np.float32)
    Wk = np.asarray(Wk, dtype=np.float32)

    query_t = np.ascontiguousarray(query.T.reshape(QC, P, B).transpose(1, 0, 2))
    wqt_h = np.ascontiguousarray(
        Wq.transpose(0, 2, 1).reshape(H, QC, P, AD).transpose(0, 2, 1, 3)
    )
    wk_h = np.ascontiguousarray(
        Wk.reshape(H, AC, P, KD).transpose(0, 2, 1, 3)
    )
    base_row = _make_base()

    return [
        _prep_core_inputs(query_t, keys, value, key_len, wqt_h, wk_h, base_row, c)
        for c in range(NCORES)
    ]


def kernel(query, keys, value, key_len, Wq, Wk):
    in_maps = make_in_maps(query, keys, value, key_len, Wq, Wk)
    nc = _get_nc()
    results = run_bass_kernel_spmd(nc, in_maps, core_ids=list(range(NCORES))).results

    ctx = np.concatenate([r["ctx"].reshape(BL, H * KD) for r in results], axis=0)
    attn = np.concatenate(
        [r["attn"].reshape(BL, H, S) for r in results], axis=0
    )
    return ctx, attn


# revision 33
# speedup vs baseline: 1.1116x; 1.0278x over previous
"""Multi-head dot-product attention (single query step) on 8 TRN2 NeuronCores.

Reference computation (B=64, S=2048, QD=1024, KD=512, AD=512, H=8):
    q      = einsum('bq,haq->bha', query, Wq)            # (B,H,AD)
    k      = einsum('bsk,hak->bhsa', keys, Wk)           # (B,H,S,AD)
    energy = einsum('bha,bhsa->bhs', q, k)               # (B,H,S)
    energy = where(s >= key_len[b], -1e10, energy)
    attn   = softmax(energy, axis=-1)                    # (B,H,S)
    ctx    = einsum('bhs,bsd->bhd', attn, value)         # (B,H,KD) -> (B, H*KD)
    return ctx.reshape(B, -1), attn

Key algebraic optimization: energy contracts q against k over AD, so Wk folds
into the query side:
    qk[b,h,c] = sum_a q[b,h,a] * Wk[h,a,c]               # (B,H,KD) - tiny
    energy[b,h,s] = sum_c qk[b,h,c] * keys[b,s,c]
This removes the (B,H,S,AD) key projection (~550 GFLOP) entirely; the kernel
becomes memory-bound on streaming keys+value once each.

Sharding: attention is data-parallel over batch (8 batches per core). The
tiny qk projection is sharded over HEADS instead (core h computes head h for
all 64 batches, so each core loads only its own Wq[h]/Wk[h] - 3MB instead of
24MB of weights), then a 128KB-per-core AllToAll redistributes qk so every
core holds all heads for its own batches.

Matmuls run in float32r (single-pass fp32, ~1.5e-4 rel err) - 4x the
streaming rate of strict fp32 on the PE.

Masking: one fused DVE op per energy tile computes
    energy' = min(base[s] + key_len[b]*1e7, energy),  base[s] = 1e6-(s+1)*1e7
valid positions keep energy exactly; masked positions become <= -9e6 whose
exp underflows to exactly 0 - same as the reference's -1e10 masked_fill.
"""

import numpy as np

import concourse.bass as bass
import concourse.bacc as bacc
import concourse.mybir as mybir
from concourse.tile import TileContext
from concourse.masks import make_identity
from concourse.bass_utils import run_bass_kernel_spmd
from contextlib import ExitStack

FP = mybir.dt.float32
FPR = mybir.dt.float32r
P = 128

B, S, QD, KD, AD, H = 64, 2048, 1024, 512, 512, 8
NCORES = 8
BL = B // NCORES          # batches per core = 8
BH = BL * H               # (b,h) rows per core = 64
QC = QD // P              # 8 chunks of query dim
AC = AD // P              # 4 chunks of attention dim
CC = KD // P              # 4 chunks of key-feature dim
SC = S // P               # 16 chunks of seq dim
SH = S // 2               # seq half


def build_kernel():
    nc = bacc.Bacc("TRN2", target_bir_lowering=False, debug=False, num_devices=NCORES)

    # All inputs are pre-laid-out on the host so every DMA is a contiguous
    # [128, X] block per partition. wq/wk hold only THIS core's head.
    queryt = nc.declare_dram_parameter("queryt", [P, QC, BL], FPR, isOutput=False)
    wq = nc.declare_dram_parameter("wq", [H, P, QC, AD], FPR, isOutput=False)
    wk = nc.declare_dram_parameter("wk", [H, P, AC, KD], FPR, isOutput=False)
    keyst = nc.declare_dram_parameter("keyst", [BL, 2, P, CC, SH], FPR, isOutput=False)
    value = nc.declare_dram_parameter(
        "value", [BL, 2, P, SC // 2, KD], FPR, isOutput=False
    )
    base = nc.declare_dram_parameter("base", [H, S], FP, isOutput=False)
    klen7 = nc.declare_dram_parameter("klen7", [H, BL], FP, isOutput=False)
    attn_out = nc.declare_dram_parameter("attn", [BH, S], FP, isOutput=True)
    ctx_out = nc.declare_dram_parameter("ctx", [BH, KD], FP, isOutput=True)

    with TileContext(nc) as tc, ExitStack() as ctx:
        persist = ctx.enter_context(tc.tile_pool(name="persist", bufs=1))
        en_pool = ctx.enter_context(tc.tile_pool(name="en", bufs=3))
        stats = ctx.enter_context(tc.tile_pool(name="st", bufs=3))
        ctxo_pool = ctx.enter_context(tc.tile_pool(name="cx", bufs=2))
        keys_pool = ctx.enter_context(tc.tile_pool(name="keys", bufs=4))
        val_pool = ctx.enter_context(tc.tile_pool(name="val", bufs=4))
        wq_pool = ctx.enter_context(tc.tile_pool(name="wq", bufs=1))
        wk_pool = ctx.enter_context(tc.tile_pool(name="wkp", bufs=2))
        q_pool = ctx.enter_context(tc.tile_pool(name="qp", bufs=1))
        psum_small = ctx.enter_context(tc.tile_pool(name="ps", bufs=3, space="PSUM"))
        psum_energy = ctx.enter_context(tc.tile_pool(name="pe", bufs=2, space="PSUM"))

        # ---- constants / persistent tiles ----
        ident = persist.tile([8, 8], FP)
        make_identity(nc, ident)
        queryt_sb = persist.tile([P, QC, BL], FPR)
        nc.sync.dma_start(queryt_sb[:], queryt[:])
        base_sb = persist.tile([H, S], FP)
        nc.sync.dma_start(base_sb[:], base[:])
        klen7_sb = persist.tile([H, BL], FP)
        nc.sync.dma_start(klen7_sb[:], klen7[:])

        qt_sb = persist.tile([P, AC, BH], FPR)      # q.T[a, (h,b)]
        qkt_sb = persist.tile([P, CC, H, BL], FPR)  # qk.T[c, (h,b)]
        attnt_sb = persist.tile([P, SC, BH], FPR)   # attn.T[s, (b,h)]

        # ---- phase 1: per head h: q_h = query @ Wq[h]^T (own batches),
        #      qk_h = q_h @ Wk[h], both transposed into qt/qkt ----
        for h in range(H):
            wq_sb = wq_pool.tile([P, QC, AD], FPR, tag="wq")
            for i in range(4):
                nc.sync.dma_start(
                    wq_sb[:, 2 * i:2 * i + 2, :], wq[h, :, 2 * i:2 * i + 2, :]
                )
            q_ps = psum_small.tile([BL, AD], FP, tag="sm")
            for qc in range(QC):
                nc.tensor.matmul(
                    q_ps[:],
                    queryt_sb[:, qc, :],       # lhsT [K=128, M=8]
                    wq_sb[:, qc, :],           # rhs  [K=128, N=512]
                    start=(qc == 0),
                    stop=(qc == QC - 1),
                )
            q_sb = q_pool.tile([BL, AD], FP, tag="q")
            nc.scalar.copy(q_sb[:], q_ps[:])
            tr_ps = psum_small.tile([P, AC, BL], FP, tag="sm")
            for ac in range(AC):
                nc.tensor.transpose(
                    tr_ps[:, ac, :], q_sb[:, ac * P:(ac + 1) * P], ident[:]
                )
            nc.vector.tensor_copy(qt_sb[:, :, h * BL:(h + 1) * BL], tr_ps[:])

            wk_sb = wk_pool.tile([P, AC, KD], FPR, tag="wk")
            for i in range(2):
                nc.sync.dma_start(
                    wk_sb[:, 2 * i:2 * i + 2, :], wk[h, :, 2 * i:2 * i + 2, :]
                )
            qk_ps = psum_small.tile([BL, KD], FP, tag="sm")
            for ac in range(AC):
                nc.tensor.matmul(
                    qk_ps[:],
                    qt_sb[:, ac, h * BL:(h + 1) * BL],   # lhsT [K=128, M=8]
                    wk_sb[:, ac, :],                      # rhs  [K=128, N=512]
                    start=(ac == 0),
                    stop=(ac == AC - 1),
                )
            qk_sb = q_pool.tile([BL, KD], FP, tag="qk")
            nc.scalar.copy(qk_sb[:], qk_ps[:])
            tr_ps2 = psum_small.tile([P, CC, BL], FP, tag="sm")
            for cc in range(CC):
                nc.tensor.transpose(
                    tr_ps2[:, cc, :], qk_sb[:, cc * P:(cc + 1) * P], ident[:]
                )
            nc.vector.tensor_copy(qkt_sb[:, :, h, :], tr_ps2[:])

        # ---- per-batch pipeline: energy -> mask -> softmax -> transpose -> ctx
        en_tiles = {}

        def energy(b):
            en_sb = en_pool.tile([H, S], FP, tag="en")
            en_tiles[b] = en_sb
            for half in range(2):
                kt_sb = keys_pool.tile([P, CC, SH], FPR, tag="kt")
                nc.sync.dma_start(kt_sb[:], keyst[b, half])
                en_ps = psum_energy.tile([H, SH], FP, tag="enp")
                for sc2 in range(2):
                    for cc in range(CC):
                        nc.tensor.matmul(
                            en_ps[:, sc2 * 512:(sc2 + 1) * 512],
                            qkt_sb[:, cc, :, b],             # lhsT [K=128, M=8] (h)
                            kt_sb[:, cc, sc2 * 512:(sc2 + 1) * 512],
                            start=(cc == 0),
                            stop=(cc == CC - 1),
                        )
                # fused mask + psum evict:
                # en = min(base[s] + klen[b]*1e7, energy)
                nc.vector.scalar_tensor_tensor(
                    en_sb[:, half * SH:(half + 1) * SH],
                    base_sb[:, half * SH:(half + 1) * SH],
                    klen7_sb[:, b:b + 1],
                    en_ps[:],
                    op0=mybir.AluOpType.add,
                    op1=mybir.AluOpType.min,
                )

        rsums = {}
        rinvs = {}

        def exp_block(b):
            # unnormalized softmax numerator; energies are O(+-45) so exp
            # cannot overflow f32 and the max-subtraction can be skipped
            # (mathematically identical softmax)
            en_sb = en_tiles[b]
            rsum = stats.tile([H, 1], FP, tag="rs")
            rsums[b] = rsum
            nc.scalar.activation(
                en_sb[:], en_sb[:], mybir.ActivationFunctionType.Exp,
                bias=0.0, accum_out=rsum[:],
            )

        def recip_block(b):
            rinv = stats.tile([H, 1], FP, tag="ri")
            rinvs[b] = rinv
            nc.vector.reciprocal(rinv[:], rsums[b][:])

        def norm_block(b):
            # normalize the attn output in place AFTER the transposes have
            # read the unnormalized exp (Tile's WAR dep orders this); ctx
            # folds 1/rsum into its psum evict instead
            en_sb = en_tiles[b]
            nc.vector.tensor_scalar_mul(en_sb[:], en_sb[:], rinvs[b][:])
            nc.sync.dma_start(attn_out[b * H:(b + 1) * H, :], en_sb[:])

        def transposes(b):
            en_sb = en_tiles[b]
            for g in range(4):
                tr_ps = psum_small.tile([P, 4, H], FP, tag="sm")
                for j in range(4):
                    sc = g * 4 + j
                    nc.tensor.transpose(
                        tr_ps[:, j, :], en_sb[:, sc * P:(sc + 1) * P], ident[:]
                    )
                nc.vector.tensor_copy(
                    attnt_sb[:, g * 4:(g + 1) * 4, b * H:(b + 1) * H], tr_ps[:]
                )

        def ctx_block(b):
            ctx_ps = psum_small.tile([BL, KD], FP, tag="sm")
            for half in range(2):
                v_sb = val_pool.tile([P, SC // 2, KD], FPR, tag="v")
                nc.sync.dma_start(v_sb[:], value[b, half])
                for sc2 in range(SC // 2):
                    sc = half * (SC // 2) + sc2
                    nc.tensor.matmul(
                        ctx_ps[:],
                        attnt_sb[:, sc, b * H:(b + 1) * H],   # lhsT [K=128, M=8]
                        v_sb[:, sc2, :],                       # rhs  [K=128, N=512]
                        start=(sc == 0),
                        stop=(sc == SC - 1),
                    )
            ctx_sb = ctxo_pool.tile([BL, KD], FP, tag="c")
            # evict with the softmax normalization folded in
            nc.scalar.mul(ctx_sb[:], ctx_ps[:], rinvs[b][:])
            nc.sync.dma_start(ctx_out[b * H:(b + 1) * H, :], ctx_sb[:])

        energy(0)
        energy(1)
        energy(2)
        exp_block(0)
        for b in range(BL):
            if b + 1 < BL:
                exp_block(b + 1)
            recip_block(b)
            transposes(b)
            ctx_block(b)
            if b + 3 < BL:
                energy(b + 3)
            norm_block(b)

    nc.compile()
    return nc


_NC = None


def _get_nc():
    global _NC
    if _NC is None:
        _NC = build_kernel()
    return _NC


def _make_base():
    s = np.arange(S, dtype=np.float64)
    row = (1e6 - (s + 1) * 1e7).astype(np.float32)
    return np.ascontiguousarray(np.tile(row, (H, 1)))


def _prep_core_inputs(query, keys, value, key_len, wqt_h, wk_h, base_row, c):
    bs = slice(c * BL, (c + 1) * BL)
    query_t = np.ascontiguousarray(
        query[bs].T.reshape(QC, P, BL).transpose(1, 0, 2)
    )
    # keyst: [BL, half, P, CC, SH] so each half-tile DMA is fully contiguous
    kt = keys[bs].transpose(0, 2, 1).reshape(BL, CC, P, 2, SH)
    kt = np.ascontiguousarray(kt.transpose(0, 3, 2, 1, 4))
    # value: [BL, half, P, SC//2, KD]
    v = value[bs].reshape(BL, 2, SC // 2, P, KD)
    v = np.ascontiguousarray(v.transpose(0, 1, 3, 2, 4))
    kl = (key_len[bs].astype(np.float64) * 1e7).astype(np.float32)
    kl = np.ascontiguousarray(np.tile(kl[None, :], (H, 1)))
    return {
        "queryt": query_t, "wq": wqt_h, "wk": wk_h,
        "keyst": kt, "value": v, "base": base_row, "klen7": kl,
    }


def make_in_maps(query, keys, value, key_len, Wq, Wk):
    query = np.asarray(query, dtype=np.float32)
    keys = np.asarray(keys, dtype=np.float32)
    value = np.asarray(value, dtype=np.float32)
    key_len = np.asarray(key_len, dtype=np.int32)
    Wq = np.asarray(Wq, dtype=np.float32)
    Wk = np.asarray(Wk, dtype=np.float32)

    wqt_h = np.ascontiguousarray(
        Wq.transpose(0, 2, 1).reshape(H, QC, P, AD).transpose(0, 2, 1, 3)
    )
    wk_h = np.ascontiguousarray(
        Wk.reshape(H, AC, P, KD).transpose(0, 2, 1, 3)
    )
    base_row = _make_base()

    return [
        _prep_core_inputs(query, keys, value, key_len, wqt_h, wk_h, base_row, c)
        for c in range(NCORES)
    ]


def kernel(query, keys, value, key_len, Wq, Wk):
    in_maps = make_in_maps(query, keys, value, key_len, Wq, Wk)
    nc = _get_nc()
    results = run_bass_kernel_spmd(nc, in_maps, core_ids=list(range(NCORES))).results

    ctx = np.concatenate([r["ctx"].reshape(BL, H * KD) for r in results], axis=0)
    attn = np.concatenate(
        [r["attn"].reshape(BL, H, S) for r in results], axis=0
    )
    return ctx, attn
